# revision 24
# baseline (speedup 1.0000x reference)
"""Trainium2 Bass kernel for nn_AttentionV2 (dense transformer attention block).

Reference computation (B=4, C=256, H=W=48, heads=8, d=32, N=2304):
  qk   = conv1x1(x, w_qk) -> q,k per head [d, N]
  v4   = conv1x1(x, w_v)
  pe   = dwconv3x3(v4, w_pe)            (depthwise, SAME)
  S    = q^T k * d^-0.5 ; P = softmax_j(S)
  out  = v @ P^T  (per head)
  y    = conv1x1(out + pe, w_proj)

Sharding: 8 cores = 4 batches x 2 spatial halves (rows 0-23 / 24-47).
Each core computes full k,v for its batch (attention rows are independent
given full k/v) -> zero collectives. The per-core x is ROLLED by y0 rows so
the SPMD program always works on "rows 0..23"; softmax/attention are
permutation-invariant in j. Halo rows for the depthwise conv are gated by
per-core 0/1 flag inputs.

Attention is computed in "ST" layout (S^T = k^T q, j on partitions):
 - PE matmul k[:,jc]^T q -> ST psum [128 j, 1152 i]
 - ACT exp(SCALE*ST) psum -> SBUF E (bf16), no max-subtraction (|S*scale|<~5)
 - PE AV matmul with lhsT = [v^T | ones] accumulated over j-chunks gives both
   the unnormalized output (rows 0..32) and the softmax denominator (row 32).

The exp stream on the Scalar engine is the hard bottleneck (~160us of
ACTIVATE at 128 lanes x 1.2GHz); everything else must hide under it:
 - 108 exp calls of N=1536 (3 full PSUM banks each, i-chunks {512,512,128}).
 - Every 3rd ST matmul runs full-128-contraction against a zero-padded q
   copy (q_pad) purely to keep the PE activity monitor from clock-gating the
   PE to 1.2GHz (32-row matmuls do not register as activity); the other ST
   matmuls are 32-row tile_position matmuls on two concurrent row bands.
 - Softmax denominators: DVE reciprocal straight off the AV psum row, then a
   GpSimd partition_broadcast (no DRAM round-trip, no Sync-queue DMA parade).
 - conv1x1 / dwconv / vT prep run as fillers inside the attention loop.
"""

import os
import numpy as np
from ml_dtypes import bfloat16

C = 256
HW = 48
N = HW * HW          # 2304
NH = N // 2          # 1152 per-core i-pixels
NHEADS = 8
D = 32
SCALE = D ** -0.5
NJ = N // 128        # 18 j-chunks
ISUBS = [(0, 512), (512, 512), (1024, 128)]
NCH384 = [(k * 384, 384) for k in range(6)]   # full-image conv col chunks
QCH = [(0, 512), (512, 512), (1024, 128)]     # q conv col chunks
V4CH = [(0, 512), (512, 512), (1024, 176)]    # v4 conv col chunks (incl row24)
WPACK = 1041

_CACHE = {}


def _build_bass():
    import concourse.bass as bass
    import concourse.bacc as bacc
    import concourse.mybir as mybir
    from concourse import tile

    f32 = mybir.dt.float32
    bf16 = mybir.dt.bfloat16
    AF = mybir.ActivationFunctionType
    OP = mybir.AluOpType

    nc = bacc.Bacc()

    x_d = nc.dram_tensor("x", [C, N], bf16, kind="ExternalInput")
    # all weights/biases packed into one tensor: [wqkT|wvT|wprojT|wpe|biases|halo]
    wpack_d = nc.dram_tensor("wpack", [C, WPACK], f32, kind="ExternalInput")
    # bf16 copy of the matmul weights [wqkT|wvT|wprojT]
    wpackh_d = nc.dram_tensor("wpackh", [C, 1024], bf16, kind="ExternalInput")
    out_d = nc.dram_tensor("out", [C, NH], f32, kind="ExternalOutput")

    with tile.TileContext(nc) as tc:
        with (
            tc.tile_pool(name="wts", bufs=1) as wp,
            tc.tile_pool(name="per", bufs=1) as pp,
        ):
            # ---- early ACT table load: exp on a dep-free scratch tile so the
            # ~2.7us table DMA overlaps the input DMAs instead of stalling the
            # first real exp call.
            scra = pp.tile([128, 16], f32, tag="scra", name="scra")
            scrb = pp.tile([128, 16], bf16, tag="scrb", name="scrb")
            nc.vector.memset(scra[:, :], 0.0)
            nc.scalar.activation(scrb[:, :], scra[:, :], AF.Exp, scale=1.0)

            # ---- persistent weight/bias tiles (single packed DMA per chunk)
            wsb = [wp.tile([128, WPACK], f32, tag=f"wsb{c}", name=f"wsb{c}") for c in range(2)]
            wph = [wp.tile([128, 1024], bf16, tag=f"wph{c}", name=f"wph{c}") for c in range(2)]
            for c in range(2):
                nc.sync.dma_start(out=wph[c][:, :], in_=wpackh_d[128 * c:128 * (c + 1), :])
            for c in range(2):
                nc.sync.dma_start(out=wsb[c][:, :], in_=wpack_d[128 * c:128 * (c + 1), :])
            wqkT = [wph[c][:, 0:512] for c in range(2)]
            wvT = [wph[c][:, 512:768] for c in range(2)]
            wprojT = [wph[c][:, 768:1024] for c in range(2)]
            wpe = [wsb[c][:, 1024:1033] for c in range(2)]
            bq = [wsb[c][:, 1033:1034] for c in range(2)]
            bk = [wsb[c][:, 1034:1035] for c in range(2)]
            bv = [wsb[c][:, 1035:1036] for c in range(2)]
            bvpe = [wsb[c][:, 1036:1037] for c in range(2)]
            bproj = [wsb[c][:, 1037:1038] for c in range(2)]
            bvht = [wsb[c][:, 1038:1039] for c in range(2)]
            halo = [wsb[c][:, 1039:1041] for c in range(2)]

            # ---- persistent activations
            q_sb = [pp.tile([128, NH], bf16, tag=f"q{c}", name=f"q{c}") for c in range(2)]
            k_sb = [pp.tile([128, N], bf16, tag=f"k{c}", name=f"k{c}") for c in range(2)]
            # zero-padded per-head q: only rows 32*(h%4)..+32 are live; a
            # full-128-contraction matmul against the SHARED k then computes
            # head h's S^T exactly (other rows contribute zero).
            q_pad = [pp.tile([128, NH], bf16, tag=f"qp{h}", name=f"qp{h}")
                     for h in range(NHEADS)]
            for h in range(NHEADS):
                nc.vector.memset(q_pad[h][:, :], 0.0)
            vT = pp.tile([128, NJ, NHEADS, 33], bf16, tag="vT", name="vT")
            v4 = [pp.tile([128, 26, HW], f32, tag=f"v4{c}", name=f"v4{c}") for c in range(2)]
            htop = [pp.tile([128, 1, HW], f32, tag=f"htop{c}", name=f"htop{c}") for c in range(2)]
            hbot = [pp.tile([128, 1, HW], f32, tag=f"hbot{c}", name=f"hbot{c}") for c in range(2)]
            pe = [pp.tile([128, 24, HW], f32, tag=f"pe{c}", name=f"pe{c}") for c in range(2)]
            outU = [pp.tile([128, NH], f32, tag=f"outU{c}", name=f"outU{c}") for c in range(2)]
            # per-head softmax 1/l rows, all on partition 0
            rl4 = pp.tile([128, 4, NH], f32, tag="rl4", name="rl4")
            rl4h = pp.tile([128, 4, NH], bf16, tag="rl4h", name="rl4h")
            # staging for l and the reciprocal scratch (partition 0, per h%2)
            rls = pp.tile([128, 2, 2, 512], f32, tag="rls", name="rls")
            ones_sb = pp.tile([128, 32], bf16, tag="ones", name="ones")
            nc.vector.memset(ones_sb[:, :], 1.0)
            proj_in = [pp.tile([128, NH], bf16, tag=f"pin{c}", name=f"pin{c}") for c in range(2)]

            nc.vector.memset(vT[:, :, :, 32:33], 1.0)

            # ================= stage 1: minimal prefix =================
            # Only what the first exp call needs runs up front: x, k chunk 0,
            # all of q (heads 0-3) + q_pad copies for heads 0/1, vT chunk 0.
            # Everything else becomes "filler" closures interleaved into the
            # attention loop so the first exp starts as early as possible.
            xpool = tc.tile_pool(name="xp", bufs=1)
            xp = xpool.__enter__()
            x_sb = [xp.tile([128, N], bf16, tag=f"x{c}", name=f"x{c}") for c in range(2)]
            for s in range(0, N, 768):
                for c in range(2):
                    nc.sync.dma_start(out=x_sb[c][:, s:s + 768],
                                      in_=x_d[128 * c:128 * (c + 1), s:s + 768])

            def qk_chunk(t, c0, w, pool):
                is_q = t < 2
                oc = t % 2
                dst = q_sb[oc] if is_q else k_sb[oc]
                bias = bq[oc] if is_q else bk[oc]
                pt = pool.tile([128, 512], f32, tag="pps", name="pps")
                for c in range(2):
                    nc.tensor.matmul(
                        pt[:, :w],
                        wqkT[c][:, 128 * t:128 * (t + 1)],
                        x_sb[c][:, c0:c0 + w],
                        start=(c == 0), stop=(c == 1),
                    )
                nc.vector.tensor_scalar(dst[:, c0:c0 + w], pt[:, :w], bias[:, :], None, OP.add)

            def qpad_copy(h, c0, w):
                r = 32 * (h % 4)
                nc.sync.dma_start(out=q_pad[h][r:r + 32, c0:c0 + w],
                                  in_=q_sb[h // 4][r:r + 32, c0:c0 + w])

            def vt_chunk(j, pool):
                pt = pool.tile([128, 512], f32, tag="pps", name="pps")
                for c in range(2):
                    nc.tensor.matmul(
                        pt[:, :256],
                        x_sb[c][:, 128 * j:128 * (j + 1)],
                        wvT[c][:, :],
                        start=(c == 0), stop=(c == 1),
                    )
                nc.vector.tensor_copy(
                    vT[:, j, :, 0:32],
                    pt[:, :256].rearrange("p (h d) -> p h d", d=32),
                )

            def v4_chunk(oc, c0, w, pool):
                pt = pool.tile([128, 512], f32, tag="pps", name="pps")
                for c in range(2):
                    nc.tensor.matmul(
                        pt[:, :w],
                        wvT[c][:, 128 * oc:128 * (oc + 1)],
                        x_sb[c][:, c0:c0 + w],
                        start=(c == 0), stop=(c == 1),
                    )
                v4f = v4[oc][:, :, :].rearrange("p a b -> p (a b)")
                nc.vector.tensor_scalar(v4f[:, c0:c0 + w], pt[:, :w], bv[oc][:, :], None, OP.add)

            def v4_halo(oc, pool):
                pt = pool.tile([128, 512], f32, tag="pps", name="pps")
                for c in range(2):
                    nc.tensor.matmul(
                        pt[:, :48],
                        wvT[c][:, 128 * oc:128 * (oc + 1)],
                        x_sb[c][:, 47 * 48:48 * 48],
                        start=(c == 0), stop=(c == 1),
                    )
                nc.scalar.activation(htop[oc][:, 0, :], pt[:, :48], AF.Identity,
                                     bias=bvht[oc][:, :], scale=halo[oc][:, 0:1])
                nc.scalar.activation(hbot[oc][:, 0, :], v4[oc][:, 24, :], AF.Copy,
                                     scale=halo[oc][:, 1:2])

            def pe_taps(oc):
                w9 = wpe[oc]
                acc = pe[oc]
                src = v4[oc]
                nc.vector.tensor_scalar(acc[:, :, :], src[:, 0:24, :], w9[:, 4:5], None, OP.mult)
                taps = [
                    (-1, -1, (1, 24), (0, 23), (1, 48), (0, 47)),
                    (-1, 0, (1, 24), (0, 23), (0, 48), (0, 48)),
                    (-1, 1, (1, 24), (0, 23), (0, 47), (1, 48)),
                    (0, -1, (0, 24), (0, 24), (1, 48), (0, 47)),
                    (0, 1, (0, 24), (0, 24), (0, 47), (1, 48)),
                    (1, -1, (0, 23), (1, 24), (1, 48), (0, 47)),
                    (1, 0, (0, 23), (1, 24), (0, 48), (0, 48)),
                    (1, 1, (0, 23), (1, 24), (0, 47), (1, 48)),
                ]
                for (dy, dx, oy, iy, ox, ix) in taps:
                    wap = w9[:, 3 * (dy + 1) + (dx + 1):3 * (dy + 1) + (dx + 1) + 1]
                    nc.vector.scalar_tensor_tensor(
                        acc[:, oy[0]:oy[1], ox[0]:ox[1]],
                        src[:, iy[0]:iy[1], ix[0]:ix[1]],
                        wap,
                        acc[:, oy[0]:oy[1], ox[0]:ox[1]],
                        OP.mult, OP.add,
                    )
                for (dx, ox, ix) in [(-1, (1, 48), (0, 47)), (0, (0, 48), (0, 48)), (1, (0, 47), (1, 48))]:
                    wap = w9[:, (dx + 1):(dx + 2)]
                    nc.vector.scalar_tensor_tensor(
                        acc[:, 0:1, ox[0]:ox[1]], htop[oc][:, :, ix[0]:ix[1]],
                        wap, acc[:, 0:1, ox[0]:ox[1]], OP.mult, OP.add,
                    )
                    wap = w9[:, 6 + (dx + 1):6 + (dx + 2)]
                    nc.vector.scalar_tensor_tensor(
                        acc[:, 23:24, ox[0]:ox[1]], hbot[oc][:, :, ix[0]:ix[1]],
                        wap, acc[:, 23:24, ox[0]:ox[1]], OP.mult, OP.add,
                    )

            with tc.tile_pool(name="ps1", bufs=2, space="PSUM") as ps1:
                qk_chunk(2, 0, 384, ps1)            # k chunk 0 (heads 0-3)
                for (c0, w) in QCH:
                    qk_chunk(0, c0, w, ps1)         # q heads 0-3 (all cols)
                qpad_copy(0, 0, NH)
                qpad_copy(1, 0, NH)
                vt_chunk(0, ps1)

            def emit_norm(oc, i0, icw, pool):
                # broadcast per-head 1/l across the 32 output rows with a K=1
                # bf16 matmul (ones[1,32]^T @ rl[1,icw] -> psum band 32g):
                # no DMA, no DRAM round-trip, handles partition offsets.
                rlb = pool.tile([128, 512], f32, tag="pps", name="pps")
                for g in range(4):
                    nc.tensor.matmul(
                        rlb[32 * g:32 * (g + 1), 0:icw],
                        ones_sb[0:1, 0:32],
                        rl4h[0:1, g, i0:i0 + icw],
                        start=True, stop=True,
                        tile_position=(0, 32 * g),
                        skip_group_check=True,
                    )
                pef = pe[oc][:, :, :].rearrange("p a b -> p (a b)")
                nc.vector.tensor_tensor(
                    proj_in[oc][:, i0:i0 + icw], outU[oc][:, i0:i0 + icw],
                    rlb[:, 0:icw], OP.mult,
                )
                nc.vector.scalar_tensor_tensor(
                    proj_in[oc][:, i0:i0 + icw], proj_in[oc][:, i0:i0 + icw],
                    bvpe[oc][:, :], pef[:, i0:i0 + icw], OP.add, OP.add,
                )

            y_sb = [pp.tile([128, NH], f32, tag=f"y{c}", name=f"y{c}") for c in range(2)]

            # ================= stage 2: attention + fillers ============
            with (
                tc.tile_pool(name="ep", bufs=4) as ep,
                tc.tile_pool(name="stp", bufs=2, space="PSUM") as stp,
                tc.tile_pool(name="ava", bufs=1, space="PSUM") as ava,
                tc.tile_pool(name="ps3", bufs=1, space="PSUM") as ps3,
            ):
                def emit_proj(i0, icw):
                    for oc in range(2):
                        pt = ps3.tile([128, 512], f32, tag="pps", name="pps")
                        for c in range(2):
                            nc.tensor.matmul(
                                pt[:, :icw],
                                wprojT[c][:, 128 * oc:128 * (oc + 1)],
                                proj_in[c][:, i0:i0 + icw],
                                start=(c == 0), stop=(c == 1),
                            )
                        nc.vector.tensor_scalar(y_sb[oc][:, i0:i0 + icw], pt[:, :icw],
                                                bproj[oc][:, :], None, OP.add)
                        nc.sync.dma_start(out=out_d[128 * oc:128 * (oc + 1), i0:i0 + icw],
                                          in_=y_sb[oc][:, i0:i0 + icw])

                # Conv fillers, spread so the PE load stays flat and well
                # under the exp stream. Deadlines: vt_j before AV(j) in group
                # 0 chunk 0; k oc1 (qk3) + q_pad h2/h3 before group 1;
                # q oc1 (qk1) + q_pad h4-7 before group 2; v4/pe(oc) before
                # emit_norm(oc) at the end of group 2*oc+1 chunk 0.
                FILL = {}
                f00 = []
                for j in range(1, 6):
                    f00.append(lambda c0=384 * j: qk_chunk(2, c0, 384, ps3))
                    f00.append(lambda j=j: vt_chunk(j, ps3))
                for j in range(6, NJ):
                    f00.append(lambda j=j: vt_chunk(j, ps3))
                FILL[(0, 0)] = f00
                FILL[(0, 1)] = ([(lambda c0=c0, w=w: qk_chunk(3, c0, w, ps3))
                                 for (c0, w) in NCH384]
                                + [lambda: qpad_copy(2, 0, NH), lambda: qpad_copy(3, 0, NH)])
                FILL[(1, 0)] = ([(lambda c0=c0, w=w: qk_chunk(1, c0, w, ps3))
                                 for (c0, w) in QCH]
                                + [(lambda c0=c0, w=w: v4_chunk(0, c0, w, ps3))
                                   for (c0, w) in V4CH]
                                + [lambda: v4_halo(0, ps3), lambda: pe_taps(0)]
                                + [lambda h=h: qpad_copy(h, 0, NH) for h in range(4, 8)])
                FILL[(2, 0)] = ([(lambda c0=c0, w=w: v4_chunk(1, c0, w, ps3))
                                 for (c0, w) in V4CH]
                                + [lambda: v4_halo(1, ps3), lambda: pe_taps(1)])

                for grp in range(4):
                    heads = [2 * grp, 2 * grp + 1]
                    oc = heads[0] // 4
                    units = []
                    for j in range(NJ):
                        for h in heads:
                            units.append((h, j))
                    for ici, (i0, icw) in enumerate(ISUBS):
                        spb = 512 // icw          # matmul slots per psum bank
                        spc = 3 * spb             # slots per exp call
                        calls = [units[spc * t:spc * (t + 1)]
                                 for t in range(len(units) // spc)]
                        fillers = FILL.get((grp, ici), [])
                        fi = 0
                        avl = ava.tile([97, 512], f32, tag="avla", name="avla")
                        ets = {}

                        def emit_st(t):
                            st = stp.tile([128, 3, 512], f32, tag="st", name="st")
                            et = ep.tile([128, 3, 512], bf16, tag="E", name="E")
                            for s, (h, j) in enumerate(calls[t]):
                                bank, off = s // spb, icw * (s % spb)
                                if s % 3 == 0 or off != 0:
                                    # full-contraction matmul vs zero-padded q:
                                    # every 3rd to keep the PE activity monitor
                                    # at full clock (32-row matmuls don't
                                    # register), and all mid-bank outputs
                                    # (tile_position matmuls abort the NEFF
                                    # when the psum out isn't bank-aligned).
                                    nc.tensor.matmul(
                                        st[:, bank, off:off + icw],
                                        k_sb[oc][:, 128 * j:128 * (j + 1)],
                                        q_pad[h][:, i0:i0 + icw],
                                        start=True, stop=True,
                                    )
                                else:
                                    r = 32 * (h % 4)
                                    nc.tensor.matmul(
                                        st[:, bank, off:off + icw],
                                        k_sb[oc][r:r + 32, 128 * j:128 * (j + 1)],
                                        q_sb[oc][r:r + 32, i0:i0 + icw],
                                        start=True, stop=True,
                                        tile_position=(r, 0),
                                    )
                            nc.scalar.activation(et[:, :, :], st[:, :, :],
                                                 AF.Exp, scale=SCALE)
                            ets[t] = et

                        def emit_av(t):
                            et = ets.pop(t)
                            for s, (h, j) in enumerate(calls[t]):
                                bank, off = s // spb, icw * (s % spb)
                                cp = 64 * (h % 2)
                                nc.tensor.matmul(
                                    avl[cp:cp + 33, 0:icw],
                                    vT[:, j, h, 0:33],
                                    et[:, bank, off:off + icw],
                                    start=(j == 0), stop=(j == NJ - 1),
                                    tile_position=(0, cp),
                                    # CoreSim's zero-region bookkeeping mis-
                                    # addresses partition-offset outputs; the
                                    # two heads' groups are genuinely disjoint
                                    # (partitions 0-32 vs 64-96).
                                    skip_group_check=True,
                                )

                        for t in range(len(calls)):
                            emit_st(t)
                            for _ in range(2):
                                if fi < len(fillers):
                                    fillers[fi]()
                                    fi += 1
                            if t >= 1:
                                emit_av(t - 1)
                        while fi < len(fillers):
                            fillers[fi]()
                            fi += 1
                        emit_av(len(calls) - 1)

                        # drain: unnormalized out rows + softmax denominator.
                        # The custom-DVE reciprocal mishandles non-zero
                        # partition bases on HW (like partition_broadcast), so
                        # stage l to partition 0 with a plain copy first and
                        # run the reciprocal entirely at partition base 0.
                        for h in heads:
                            g = h % 4
                            hh = h % 2
                            cp = 64 * hh
                            nc.vector.tensor_copy(outU[oc][32 * g:32 * (g + 1), i0:i0 + icw],
                                                  avl[cp:cp + 32, 0:icw])
                            nc.vector.tensor_copy(rls[0:1, hh, 0, 0:icw],
                                                  avl[cp + 32:cp + 33, 0:icw])
                            nc.vector.reciprocal_approx_accurate(
                                rl4[0:1, g, i0:i0 + icw],
                                rls[0:1, hh, 0, 0:icw],
                                rls[0:1, hh, 1, 0:icw],
                            )
                            nc.vector.tensor_copy(rl4h[0:1, g, i0:i0 + icw],
                                                  rl4[0:1, g, i0:i0 + icw])
                        if grp % 2 == 1:
                            emit_norm(oc, i0, icw, ps3)
                        if grp == 3:
                            emit_proj(i0, icw)
            xpool.__exit__(None, None, None)

    nc.finalize()
    return nc


def _prep_inputs(x, w_qk, b_qk, w_v, b_v, w_pe, b_pe, w_proj, b_proj):
    f = np.float32
    base = np.zeros((C, WPACK), dtype=f)
    # reference reshapes qk conv output to (h, 2d): channels 64h..64h+32 are
    # q_h, 64h+32..64h+64 are k_h. Repack host-side to [q by head | k by head].
    wqk2 = w_qk[:, :, 0, 0].reshape(NHEADS, 2 * D, C)
    bqk2 = b_qk.reshape(NHEADS, 2 * D)
    wq = wqk2[:, :D].reshape(C, C)
    wk = wqk2[:, D:].reshape(C, C)
    base[:, 0:256] = wq.T
    base[:, 256:512] = wk.T
    base[:, 512:768] = w_v[:, :, 0, 0].T
    base[:, 768:1024] = w_proj[:, :, 0, 0].T
    base[:, 1024:1033] = w_pe[:, 0].reshape(C, 9)
    base[:, 1033] = bqk2[:, :D].reshape(C)
    base[:, 1034] = bqk2[:, D:].reshape(C)
    base[:, 1035] = b_v
    base[:, 1036] = b_v + b_pe
    base[:, 1037] = b_proj
    wpackh = np.ascontiguousarray(base[:, 0:1024].astype(bfloat16))

    in_maps = []
    for core in range(8):
        b, half = core // 2, core % 2
        y0 = 24 * half
        xb = x[b].reshape(C, HW, HW).astype(f)
        xr = np.concatenate([xb[:, y0:, :], xb[:, :y0, :]], axis=1)
        halo_top = 1.0 if half == 1 else 0.0
        halo_bot = 1.0 if half == 0 else 0.0
        wpack = base.copy()
        wpack[:, 1038] = halo_top * b_v
        wpack[:, 1039] = halo_top
        wpack[:, 1040] = halo_bot
        in_maps.append({
            "x": np.ascontiguousarray(xr.reshape(C, N).astype(bfloat16)),
            "wpack": wpack, "wpackh": wpackh,
        })
    return in_maps


def kernel(**inputs):
    from concourse.bass_utils import run_bass_kernel_spmd

    if "nc" not in _CACHE:
        _CACHE["nc"] = _build_bass()
    nc = _CACHE["nc"]

    in_maps = _prep_inputs(**inputs)
    res = run_bass_kernel_spmd(nc, in_maps, core_ids=list(range(8)))
    y = np.empty((4, C, HW, HW), dtype=np.float32)
    for core in range(8):
        b, half = core // 2, core % 2
        y0 = 24 * half
        y[b][:, y0:y0 + 24, :] = res.results[core]["out"].reshape(C, 24, HW)
    return y
